# revision 23
# baseline (speedup 1.0000x reference)
"""Trainium2 Bass kernel for nn_MultiHeadAttention_79706003079680.

Reference (fp32):
    qp = (q @ Wq + bq) * SCALE      # [B, N, PROJ]
    kp = k @ Wk + bk
    vp = v @ Wv + bv
    scores = einsum('bnd,bmd->bnm', qp, kp)
    attn = softmax(scores, axis=1)          # over the QUERY axis n
    x = einsum('bnm,bmd->bnd', attn, vp)
    out = x @ Wo + bo                       # [B, N, HIDDEN]

Sharding: 8 cores = 4 batches x 2 key-halves (m in [mh*1024, mh*1024+1024)).
Softmax over n couples all queries for a fixed key m, so each core keeps
all n=2048 queries and a slice of keys. Each core emits a partial
out^T [HIDDEN, N]; the host sums the two key-halves per batch, transposes,
and adds bo.

Single-pass structure (everything SBUF-resident, no DRAM round-trips):
  K:  kp^T [P, DB, M] f16 resident (Wk fp32r, bias via ACT drain).
  AB: per 512-wide n-chunk: project q chunk (wq16 f16 resident, qp chunk
      f16 in SBUF only) then scores^T for all 8 m-blocks at N=512 free dim.
      exp() is applied directly on the PSUM drain with a constant -40 bias
      (softmax normalizer is deferred: e' = exp(s-40) and Z' = sum_n e'
      cancel in e'/Z', so no per-column max pass is needed).
  V:  vp = (v @ Wv + bv) * (1/Z') folded into the ACT drain (scale=rZ AP),
      bf16 resident.
  C:  x^T = vp^T @ e per d-block (f16), out^T = Wo^T @ x^T, DMA to DRAM.

All big matmuls run at 1 PE cycle/row and 512-wide moving operands:
float32r (fp32 truncated to FP22) for the k/v projections, f16/bf16 for
q-projection / scores / x / out.
"""

import numpy as np

import concourse.bass as bass
import concourse.mybir as mybir
import concourse.tile as tile
from concourse.masks import make_identity

P = 128
HIDDEN = 512
NUM_HEADS = 8
PROJ = NUM_HEADS * HIDDEN          # 4096
B, N = 4, 2048
M = N // 2                         # keys per core = 1024
SCALE = (HIDDEN // NUM_HEADS) ** -0.5

HB = HIDDEN // P                   # 4 h-blocks of 128
DB = PROJ // P                     # 32 d-blocks of 128
NB = N // 512                      # 4 n-chunks of 512
MB = M // P                        # 8 m-blocks of 128
EXP_SHIFT = -40.0                  # constant exp bias; cancels in e/Z

F32 = mybir.dt.float32
F32R = mybir.dt.float32r
F16 = mybir.dt.float16
BF16 = mybir.dt.bfloat16
AX = mybir.AxisListType.X
AF = mybir.ActivationFunctionType


MAX_WAITS = 1


def split_excess_waits(nc, max_waits=MAX_WAITS):
    """Move excess per-instruction sem waits onto same-engine NoOps.

    This walrus build rejects instructions carrying more than a couple of
    sync-wait commands ("Too many sync wait commands" in setupSyncWait).
    A NoOp placed immediately before the instruction on the same engine
    enforces the wait in program order with identical semantics.
    """
    n_extra = 0
    for f in nc.m.functions:
        for bb in f.blocks:
            insts = bb.instructions
            i = 0
            while i < len(insts):
                inst = insts[i]
                si = getattr(inst, "sync_info", None)
                if si is not None and si.on_wait and len(si.on_wait) > max_waits:
                    waits = list(si.on_wait)
                    si.on_wait = waits[: max_waits]
                    for w in waits[max_waits:]:
                        n_extra += 1
                        nop = mybir.InstNoOp(
                            name=f"I-wsplit{n_extra}",
                            ins=[],
                            outs=[],
                            engine=inst.engine,
                        )
                        nop.sync_info = mybir.SyncInfo(on_wait=[w], on_update=[])
                        try:
                            nc.register_instruction(nop)
                        except Exception:
                            pass
                        # insert immediately before inst (inst shifts right)
                        insts.insert(i, nop)
                        i += 1
                i += 1
    return n_extra


class PatchedTC(tile.TileContext):
    """TileContext that post-processes the module to satisfy this walrus
    build's per-instruction sync-wait limit."""

    def __exit__(self, exc_type, exc_val, exc_tb):
        ret = super().__exit__(exc_type, exc_val, exc_tb)
        if exc_type is None:
            split_excess_waits(self.nc)
        return ret


def r(ap):
    return ap.bitcast(F32R)


def _phase_k(nc, tc, pst, psm, kb, Wk, kpT, bks, ident):
    """kp^T projection -> SBUF f16 (Wk streamed in 1024-wide quarters)."""
    with (
        tc.tile_pool(name="kph", bufs=1) as kph,
        tc.tile_pool(name="wkq", bufs=2) as wkq,
        tc.tile_pool(name="kld", bufs=3) as kld,
    ):
        kT = kph.tile([P, HB, M], F32, tag="kT")
        wk_src = Wk.ap().rearrange("(hb p) d -> p hb d", p=P).bitcast(F32R)
        DQ = 512
        wk_first = wkq.tile([P, HB, DQ], F32, tag="wk")
        nc.sync.dma_start(out=wk_first.bitcast(F32R), in_=wk_src[:, :, 0:DQ])
        for mt in range(M // P):
            k_t = kld.tile([P, HIDDEN], F32, tag="ld")
            nc.sync.dma_start(out=k_t, in_=kb[mt * P : (mt + 1) * P, :])
            for hb in range(HB):
                pt = pst.tile([P, P], F32, tag="tp")
                nc.tensor.transpose(
                    pt.bitcast(F32R),
                    k_t[:, hb * P : (hb + 1) * P].bitcast(F32R),
                    ident.bitcast(F32R),
                )
                nc.vector.tensor_copy(
                    kT[:, hb, mt * P : (mt + 1) * P].bitcast(F32R), pt.bitcast(F32R)
                )
        for dsl in range(PROJ // DQ):
            if dsl == 0:
                wk_t = wk_first
            else:
                wk_t = wkq.tile([P, HB, DQ], F32, tag="wk")
                nc.sync.dma_start(
                    out=wk_t.bitcast(F32R),
                    in_=wk_src[:, :, dsl * DQ : (dsl + 1) * DQ],
                )
            for db2 in range(DQ // P):
                db = dsl * (DQ // P) + db2
                for m2 in range(M // 512):
                    ps = psm.tile([P, 512], F32, tag="mm")
                    for hb in range(HB):
                        nc.tensor.matmul(
                            ps,
                            r(wk_t[:, hb, db2 * P : (db2 + 1) * P]),
                            r(kT[:, hb, m2 * 512 : (m2 + 1) * 512]),
                            start=(hb == 0),
                            stop=(hb == HB - 1),
                        )
                    nc.scalar.activation(
                        kpT[:, db, m2 * 512 : (m2 + 1) * 512],
                        ps,
                        AF.Identity,
                        bias=bks[:, db : db + 1],
                        scale=1.0,
                    )


def _emit_wq_cast(nc, tc, Wq, wq16):
    """Load Wq fp32 in 512-wide slices, cast to f16 on DVE (runs under K)."""
    with tc.tile_pool(name="wqld", bufs=2) as wqld:
        wq_src = Wq.ap().rearrange("(hb p) d -> p hb d", p=P).bitcast(F32R)
        W = 256
        for dsl in range(PROJ // W):
            wt = wqld.tile([P, HB, W], F32, tag="wq")
            nc.sync.dma_start(
                out=wt.bitcast(F32R), in_=wq_src[:, :, dsl * W : (dsl + 1) * W]
            )
            nc.vector.tensor_copy(wq16[:, :, dsl * W : (dsl + 1) * W], wt)


def _phase_ab(nc, tc, pst, psm, qb, vb, Wq, Wv, wq16, kpT, e, zp, bqs, vT, wv0,
              ident, eshift):
    """Fused q-projection + scores + exp per 512-wide n-chunk.

    The qp chunk buffer is single: PE program order is
    [qproj(i) | scores(i) | qproj(i+1) | ...], so by the time qproj(i+1)'s
    ACT drains overwrite qp[:, db, :], scores(i) has finished reading it.
    """
    with (
        tc.tile_pool(name="qpp", bufs=1) as qpp,
        tc.tile_pool(name="qld", bufs=2) as qld,
        tc.tile_pool(name="qTp", bufs=1) as qTp,
        tc.tile_pool(name="vld", bufs=2) as vld,
    ):
        def emit_qT(nb):
            qT = qTp.tile([P, HB, 512], F16, tag="qT")
            for nt in range(4):
                q_t = qld.tile([P, HIDDEN], F32, tag="ld")
                nc.sync.dma_start(
                    out=q_t, in_=qb[nb * 512 + nt * P : nb * 512 + (nt + 1) * P, :]
                )
                for hb in range(HB):
                    pt = pst.tile([P, P], F32, tag="tp")
                    nc.tensor.transpose(
                        pt.bitcast(F32R),
                        q_t[:, hb * P : (hb + 1) * P].bitcast(F32R),
                        ident.bitcast(F32R),
                    )
                    nc.vector.tensor_copy(
                        qT[:, hb, nt * P : (nt + 1) * P], pt
                    )
            return qT

        # Wq casts first in DVE program order so they drain during phase K
        # (qT copies would head-of-line-block DVE until K's matmuls finish).
        # The weight stream rides the ACT hwdge queue, q tiles ride SP.
        _emit_wq_cast(nc, tc, Wq, wq16)
        qT = emit_qT(0)
        qp = qpp.tile([P, DB, 512], F16)
        for nb in range(NB):
            # q-projection for this chunk: qp[:, db, :] = (qT @ wq16)*SCALE+bqs
            for db in range(DB):
                ps = psm.tile([P, 512], F32, tag="mm")
                for hb in range(HB):
                    nc.tensor.matmul(
                        ps,
                        wq16[:, hb, db * P : (db + 1) * P],
                        qT[:, hb, :],
                        start=(hb == 0),
                        stop=(hb == HB - 1),
                    )
                nc.scalar.activation(
                    qp[:, db, :], ps, AF.Identity,
                    bias=bqs[:, db : db + 1], scale=SCALE,
                )
            # next chunk's transposes (uses PE briefly; DMA has a full
            # scores window to land); the last two chunks pull v in for
            # phase V so its transposes hide under the scores matmuls
            if nb + 1 < NB:
                qT = emit_qT(nb + 1)
            if nb == 2:
                wv_src = Wv.ap().rearrange("(hb p) d -> p hb d", p=P).bitcast(F32R)
                nc.sync.dma_start(out=wv0.bitcast(F32R), in_=wv_src[:, :, 0:512])
            if nb >= 2:
                for mt in range(4):
                    mtt = (nb - 2) * 4 + mt
                    v_t = vld.tile([P, HIDDEN], F32, tag="ld")
                    nc.sync.dma_start(out=v_t, in_=vb[mtt * P : (mtt + 1) * P, :])
                    for hb in range(HB):
                        pt = pst.tile([P, P], F32, tag="tp")
                        nc.tensor.transpose(
                            pt.bitcast(F32R),
                            v_t[:, hb * P : (hb + 1) * P].bitcast(F32R),
                            ident.bitcast(F32R),
                        )
                        nc.vector.tensor_copy(
                            vT[:, hb, mtt * P : (mtt + 1) * P].bitcast(F32R),
                            pt.bitcast(F32R),
                        )
            # scores^T: e[:, mb, nchunk] = exp(kpT^T(d) @ qp - 40)
            for mb in range(MB):
                ps = psm.tile([P, 512], F32, tag="mm")
                for db in range(DB):
                    nc.tensor.matmul(
                        ps,
                        kpT[:, db, mb * P : (mb + 1) * P],
                        qp[:, db, :],
                        start=(db == 0),
                        stop=(db == DB - 1),
                    )
                nc.scalar.activation(
                    e[:, mb, nb * 512 : (nb + 1) * 512],
                    ps,
                    AF.Exp,
                    bias=eshift,
                    scale=1.0,
                    accum_out=zp[:, mb, nb : nb + 1],
                )


def _phase_v(nc, tc, pst, psm, Wv, bv, vT, vp, zp, Zt, rZ, wv0):
    """vp[:, mb, :] = (vT^T @ Wv + bv) * rZ[mb]  (bf16, SBUF-resident)."""
    with (
        tc.tile_pool(name="wvq", bufs=2) as wvq,
        tc.tile_pool(name="brow", bufs=1) as brow,
    ):
        for mb in range(MB):
            nc.vector.reduce_sum(Zt[:, mb : mb + 1], zp[:, mb, :], axis=AX)
        nc.vector.reciprocal(rZ, Zt)

        bvrow = brow.tile([1, PROJ], F32)
        nc.sync.dma_start(
            out=bvrow.bitcast(F32R),
            in_=bv.ap().rearrange("(o a) -> o a", o=1).bitcast(F32R),
        )
        ones_tmp = brow.tile([1, P], F32)
        nc.vector.memset(ones_tmp, 1.0)
        ones_row = brow.tile([1, P], F32)
        nc.vector.tensor_copy(ones_row.bitcast(F32R), ones_tmp.bitcast(F32R))

        wv_src = Wv.ap().rearrange("(hb p) d -> p hb d", p=P).bitcast(F32R)
        W = 512
        for dsl in range(PROJ // W):
            if dsl == 0:
                wv_t = wv0
            else:
                wv_t = wvq.tile([P, HB, W], F32, tag="wv")
                nc.sync.dma_start(
                    out=wv_t.bitcast(F32R), in_=wv_src[:, :, dsl * W : (dsl + 1) * W]
                )
            for ds2 in range(W // 512):
                dlo = dsl * W + ds2 * 512
                for mb in range(MB):
                    ps = psm.tile([P, 512], F32, tag="mm")
                    for hb in range(HB):
                        nc.tensor.matmul(
                            ps,
                            r(vT[:, hb, mb * P : (mb + 1) * P]),
                            r(wv_t[:, hb, ds2 * 512 : (ds2 + 1) * 512]),
                            start=(hb == 0),
                            stop=False,
                        )
                    nc.tensor.matmul(
                        ps,
                        r(ones_row),
                        r(bvrow[:, dlo : dlo + 512]),
                        start=False,
                        stop=True,
                    )
                    nc.scalar.activation(
                        vp[:, mb, dlo : dlo + 512],
                        ps,
                        AF.Identity,
                        scale=rZ[:, mb : mb + 1],
                    )


def _phase_c(nc, tc, psm, Wo, vp, e, wo16, outT):
    """x^T = vp^T @ e per d-block, out^T = Wo^T @ x^T -> DRAM.

    Like qp in phase AB, x_s is a single buffer: PE order is
    [x(nb) | out(nb) | x(nb+1) | ...] so out(nb) has finished reading
    x_s before x(nb+1)'s drains overwrite it. The Wo f32->f16 cast is
    emitted first; its DMA + DVE copies hide under the first x block.
    """
    with (
        tc.tile_pool(name="wold", bufs=2) as wold,
        tc.tile_pool(name="xsp", bufs=1) as xsp,
        tc.tile_pool(name="osp", bufs=2) as osp,
    ):
        wo_src = Wo.ap().rearrange("(db p) h -> p db h", p=P).bitcast(F32R)
        for wsl in range(8):
            wo_t = wold.tile([P, 4, HIDDEN], F32, tag="wo")
            nc.sync.dma_start(
                out=wo_t.bitcast(F32R), in_=wo_src[:, wsl * 4 : (wsl + 1) * 4, :]
            )
            nc.vector.tensor_copy(wo16[:, wsl * 4 : (wsl + 1) * 4, :], wo_t)

        x_s = xsp.tile([P, DB, 512], F16, tag="x")
        for nb in range(NB):
            for db in range(DB):
                ps = psm.tile([P, 512], F32, tag="mm")
                for mch in range(MB):
                    nc.tensor.matmul(
                        ps,
                        vp[:, mch, db * P : (db + 1) * P],
                        e[:, mch, nb * 512 : (nb + 1) * 512],
                        start=(mch == 0),
                        stop=(mch == MB - 1),
                    )
                nc.vector.tensor_copy(x_s[:, db, :], ps)
            for hb in range(HB):
                if nb == NB - 1 and hb == HB - 1:
                    # split the final chain so its drain+store pipelines
                    # instead of sitting fully after the last matmul
                    for nh in range(2):
                        ps2 = psm.tile([P, 512], F32, tag="mm")
                        for db in range(DB):
                            nc.tensor.matmul(
                                ps2[:, 0:256],
                                wo16[:, db, hb * P : (hb + 1) * P],
                                x_s[:, db, nh * 256 : (nh + 1) * 256],
                                start=(db == 0),
                                stop=(db == DB - 1),
                            )
                        ot = osp.tile([P, 512], F32, tag="ot")
                        nc.vector.tensor_copy(ot[:, 0:256], ps2[:, 0:256])
                        nc.sync.dma_start(
                            out=outT[
                                hb * P : (hb + 1) * P,
                                nb * 512 + nh * 256 : nb * 512 + (nh + 1) * 256,
                            ],
                            in_=ot[:, 0:256],
                        )
                    continue
                ps2 = psm.tile([P, 512], F32, tag="mm")
                for db in range(DB):
                    nc.tensor.matmul(
                        ps2,
                        wo16[:, db, hb * P : (hb + 1) * P],
                        x_s[:, db, :],
                        start=(db == 0),
                        stop=(db == DB - 1),
                    )
                ot = osp.tile([P, 512], F32, tag="ot")
                nc.vector.tensor_copy(ot, ps2)
                nc.sync.dma_start(
                    out=outT[hb * P : (hb + 1) * P, nb * 512 : (nb + 1) * 512],
                    in_=ot,
                )


def build_nc():
    nc = bass.Bass("TRN2", target_bir_lowering=False, debug=False, num_devices=8)

    qb = nc.dram_tensor("qb", [N, HIDDEN], F32, kind="ExternalInput")
    kb = nc.dram_tensor("kb", [M, HIDDEN], F32, kind="ExternalInput")
    vb = nc.dram_tensor("vb", [M, HIDDEN], F32, kind="ExternalInput")
    Wq = nc.dram_tensor("Wq", [HIDDEN, PROJ], F32, kind="ExternalInput")
    Wk = nc.dram_tensor("Wk", [HIDDEN, PROJ], F32, kind="ExternalInput")
    Wv = nc.dram_tensor("Wv", [HIDDEN, PROJ], F32, kind="ExternalInput")
    Wo = nc.dram_tensor("Wo", [PROJ, HIDDEN], F32, kind="ExternalInput")
    bq = nc.dram_tensor("bq", [PROJ], F32, kind="ExternalInput")
    bk = nc.dram_tensor("bk", [PROJ], F32, kind="ExternalInput")
    bv = nc.dram_tensor("bv", [PROJ], F32, kind="ExternalInput")
    outT = nc.dram_tensor("outT", [HIDDEN, N], F32, kind="ExternalOutput")

    with PatchedTC(nc) as tc:
        with (
            tc.tile_pool(name="singles", bufs=1) as singles,
            tc.tile_pool(name="pst", bufs=2, space="PSUM") as pst,
            tc.tile_pool(name="psm", bufs=6, space="PSUM") as psm,
        ):
            ident = singles.tile([P, P], F32)
            make_identity(nc, ident)
            # biases need (p, db) layout with d inner on partitions; a direct
            # strided DMA would be 4096 4-byte descriptors, so load the
            # contiguous [DB, P] view and PE-transpose it instead.
            bq_raw = singles.tile([DB, P], F32)
            nc.sync.dma_start(out=bq_raw, in_=bq.ap().rearrange("(a b) -> a b", b=P))
            bqs = singles.tile([P, DB], F32)
            ptb = pst.tile([P, DB], F32, tag="tp")
            nc.tensor.transpose(ptb, bq_raw, ident[:DB, :DB])
            nc.scalar.activation(bqs, ptb, AF.Identity, scale=SCALE)
            bk_raw = singles.tile([DB, P], F32)
            nc.sync.dma_start(out=bk_raw, in_=bk.ap().rearrange("(a b) -> a b", b=P))
            bks = singles.tile([P, DB], F32)
            ptb2 = pst.tile([P, DB], F32, tag="tp")
            nc.tensor.transpose(ptb2, bk_raw, ident[:DB, :DB])
            nc.vector.tensor_copy(bks, ptb2)
            zp = singles.tile([P, MB, NB], F32)
            Zt = singles.tile([P, MB], F32)
            rZ = singles.tile([P, MB], F32)
            eshift = singles.tile([P, 1], F32)
            nc.vector.memset(eshift, EXP_SHIFT)

            # e (bf16 scores-exp) lives from AB through C.
            with tc.tile_pool(name="e_lvl", bufs=1) as e_lvl:
                e = e_lvl.tile([P, MB, N], BF16)
                vT = e_lvl.tile([P, HB, M], F32)
                wv0 = e_lvl.tile([P, HB, 512], F32)
                with tc.tile_pool(name="ab", bufs=1) as ab:
                    kpT = ab.tile([P, DB, M], F16)
                    wq16 = ab.tile([P, HB, PROJ], F16)
                    _phase_k(nc, tc, pst, psm, kb, Wk, kpT, bks, ident)
                    _phase_ab(
                        nc, tc, pst, psm, qb, vb, Wq, Wv, wq16, kpT, e, zp, bqs,
                        vT, wv0, ident, eshift,
                    )
                with tc.tile_pool(name="vc", bufs=1) as vc:
                    vp = vc.tile([P, MB, PROJ], BF16)
                    _phase_v(nc, tc, pst, psm, Wv, bv, vT, vp, zp, Zt, rZ, wv0)
                    wo16 = vc.tile([P, DB, HIDDEN], F16)
                    _phase_c(nc, tc, psm, Wo, vp, e, wo16, outT)
    # A handful of waits are attached after the TileContext's own exit
    # processing; sweep again until the module is clean.
    while split_excess_waits(nc):
        pass
    return nc


class _Runner:
    """Compile the Bass program once; re-execute cheaply on later calls.

    Mirrors bass2jax.run_bass_via_pjrt's multi-core path, but keeps the
    jitted shard_map callable so repeated kernel() calls skip the
    multi-minute neuronxcc compile.
    """

    def __init__(self):
        import jax
        from jax.sharding import Mesh, PartitionSpec
        from jax.experimental.shard_map import shard_map
        from concourse import bass2jax
        import concourse.mybir as mb

        self.jax = jax
        nc = build_nc()
        self.nc = nc
        bass2jax.install_neuronx_cc_hook()

        in_names, out_names, out_avals, zero_outs = [], [], [], []
        partition_name = (
            nc.partition_id_tensor.name if nc.partition_id_tensor else None
        )
        for alloc in nc.m.functions[0].allocations:
            if not isinstance(alloc, mb.MemoryLocationSet):
                continue
            name = alloc.memorylocations[0].name
            if alloc.kind == "ExternalInput":
                if name != partition_name:
                    in_names.append(name)
            elif alloc.kind == "ExternalOutput":
                shape = tuple(alloc.tensor_shape)
                dtype = mb.dt.np(alloc.dtype)
                out_names.append(name)
                out_avals.append(jax.core.ShapedArray(shape, dtype))
                zero_outs.append(np.zeros(shape, dtype))
        n_params = len(in_names)
        n_outs = len(out_avals)
        all_in_names = list(in_names) + list(out_names)
        if partition_name is not None:
            all_in_names.append(partition_name)
        self.in_names = in_names
        self.out_names = out_names
        self.zero_outs = zero_outs

        def _body(*args):
            operands = list(args)
            if partition_name is not None:
                operands.append(bass2jax.partition_id_tensor())
            outs = bass2jax._bass_exec_p.bind(
                *operands,
                out_avals=tuple(out_avals),
                in_names=tuple(all_in_names),
                out_names=tuple(out_names),
                lowering_input_output_aliases=(),
                sim_require_finite=True,
                sim_require_nnan=True,
                nc=nc,
            )
            return tuple(outs)

        devices = jax.devices()[:8]
        mesh = Mesh(np.asarray(devices), ("core",))
        self.mesh = mesh
        in_specs = (PartitionSpec("core"),) * (n_params + n_outs)
        out_specs = (PartitionSpec("core"),) * n_outs
        self.body = _body
        self.in_specs = in_specs
        self.out_specs = out_specs
        donate = tuple(range(n_params, n_params + n_outs))
        self.sharded = jax.jit(
            shard_map(
                _body,
                mesh=mesh,
                in_specs=in_specs,
                out_specs=out_specs,
                check_rep=False,
            ),
            donate_argnums=donate,
            keep_unused=True,
        )
        self.out_avals = out_avals

    def prepare(self, in_maps):
        """Concatenate per-core inputs along axis 0 (device-shardable)."""
        return [
            np.concatenate([in_maps[c][name] for c in range(8)], axis=0)
            for name in self.in_names
        ]

    def run(self, concat_in):
        zeros = [
            np.zeros((8 * z.shape[0], *z.shape[1:]), z.dtype) for z in self.zero_outs
        ]
        out_arrs = self.sharded(*concat_in, *zeros)
        res = []
        for c in range(8):
            res.append(
                {
                    name: np.asarray(out_arrs[i]).reshape(
                        8, *self.out_avals[i].shape
                    )[c]
                    for i, name in enumerate(self.out_names)
                }
            )
        return res


_RUNNER = None


def _get_runner():
    global _RUNNER
    if _RUNNER is None:
        _RUNNER = _Runner()
    return _RUNNER


def make_in_maps(inputs):
    f32 = lambda x: np.ascontiguousarray(np.asarray(x, dtype=np.float32))
    q, k, v = f32(inputs["q"]), f32(inputs["k"]), f32(inputs["v"])
    Wq, Wk, Wv, Wo = (f32(inputs[n]) for n in ("Wq", "Wk", "Wv", "Wo"))
    bq, bk, bv = (f32(inputs[n]) for n in ("bq", "bk", "bv"))
    in_maps = []
    for c in range(8):
        b, mh = c // 2, c % 2
        sl = slice(mh * M, (mh + 1) * M)
        in_maps.append(
            {
                "qb": q[b],
                "kb": np.ascontiguousarray(k[b, sl]),
                "vb": np.ascontiguousarray(v[b, sl]),
                "Wq": Wq, "Wk": Wk, "Wv": Wv, "Wo": Wo,
                "bq": bq, "bk": bk, "bv": bv,
            }
        )
    return in_maps


def assemble_out(results, bo):
    out = np.empty((B, N, HIDDEN), dtype=np.float32)
    for b in range(B):
        acc = results[2 * b]["outT"] + results[2 * b + 1]["outT"]
        out[b] = acc.T + bo[None, :]
    return out


def kernel(**inputs):
    runner = _get_runner()
    res = runner.run(runner.prepare(make_in_maps(inputs)))
    bo = np.asarray(inputs["bo"], dtype=np.float32)
    return assemble_out(res, bo)


# revision 24
# speedup vs baseline: 2.0239x; 2.0239x over previous
"""Trainium2 Bass kernel for nn_MultiHeadAttention_79706003079680.

Reference (fp32):
    qp = (q @ Wq + bq) * SCALE      # [B, N, PROJ]
    kp = k @ Wk + bk
    vp = v @ Wv + bv
    scores = einsum('bnd,bmd->bnm', qp, kp)
    attn = softmax(scores, axis=1)          # over the QUERY axis n
    x = einsum('bnm,bmd->bnd', attn, vp)
    out = x @ Wo + bo                       # [B, N, HIDDEN]

Sharding: 8 cores = 4 batches x 2 key-halves (m in [mh*1024, mh*1024+1024)).
Softmax over n couples all queries for a fixed key m, so each core keeps
all n=2048 queries and a slice of keys. Each core emits a partial
out^T [HIDDEN, N]; the host sums the two key-halves per batch, transposes,
and adds bo.

Single-pass structure (everything SBUF-resident, no DRAM round-trips):
  K:  kp^T [P, DB, M] f16 resident (Wk fp32r, bias via ACT drain).
  AB: per 512-wide n-chunk: project q chunk (wq16 f16 resident, qp chunk
      f16 in SBUF only) then scores^T for all 8 m-blocks at N=512 free dim.
      exp() is applied directly on the PSUM drain with a constant -40 bias
      (softmax normalizer is deferred: e' = exp(s-40) and Z' = sum_n e'
      cancel in e'/Z', so no per-column max pass is needed).
  V:  vp = (v @ Wv + bv) * (1/Z') folded into the ACT drain (scale=rZ AP),
      bf16 resident.
  C:  x^T = vp^T @ e per d-block (f16), out^T = Wo^T @ x^T, DMA to DRAM.

All big matmuls run at 1 PE cycle/row and 512-wide moving operands:
float32r (fp32 truncated to FP22) for the k/v projections, f16/bf16 for
q-projection / scores / x / out.
"""

import numpy as np

import concourse.bass as bass
import concourse.mybir as mybir
import concourse.tile as tile
from concourse.masks import make_identity

P = 128
HIDDEN = 512
NUM_HEADS = 8
PROJ = NUM_HEADS * HIDDEN          # 4096
B, N = 4, 2048
M = N // 2                         # keys per core = 1024
SCALE = (HIDDEN // NUM_HEADS) ** -0.5

HB = HIDDEN // P                   # 4 h-blocks of 128
DB = PROJ // P                     # 32 d-blocks of 128
NB = N // 512                      # 4 n-chunks of 512
MB = M // P                        # 8 m-blocks of 128
EXP_SHIFT = -40.0                  # constant exp bias; cancels in e/Z

F32 = mybir.dt.float32
F32R = mybir.dt.float32r
F16 = mybir.dt.float16
BF16 = mybir.dt.bfloat16
AX = mybir.AxisListType.X
AF = mybir.ActivationFunctionType


MAX_WAITS = 1


def split_excess_waits(nc, max_waits=MAX_WAITS):
    """Move excess per-instruction sem waits onto same-engine NoOps.

    This walrus build rejects instructions carrying more than a couple of
    sync-wait commands ("Too many sync wait commands" in setupSyncWait).
    A NoOp placed immediately before the instruction on the same engine
    enforces the wait in program order with identical semantics.
    """
    n_extra = 0
    for f in nc.m.functions:
        for bb in f.blocks:
            insts = bb.instructions
            i = 0
            while i < len(insts):
                inst = insts[i]
                si = getattr(inst, "sync_info", None)
                if si is not None and si.on_wait and len(si.on_wait) > max_waits:
                    waits = list(si.on_wait)
                    si.on_wait = waits[: max_waits]
                    for w in waits[max_waits:]:
                        n_extra += 1
                        nop = mybir.InstNoOp(
                            name=f"I-wsplit{n_extra}",
                            ins=[],
                            outs=[],
                            engine=inst.engine,
                        )
                        nop.sync_info = mybir.SyncInfo(on_wait=[w], on_update=[])
                        try:
                            nc.register_instruction(nop)
                        except Exception:
                            pass
                        # insert immediately before inst (inst shifts right)
                        insts.insert(i, nop)
                        i += 1
                i += 1
    return n_extra


class PatchedTC(tile.TileContext):
    """TileContext that post-processes the module to satisfy this walrus
    build's per-instruction sync-wait limit."""

    def __exit__(self, exc_type, exc_val, exc_tb):
        ret = super().__exit__(exc_type, exc_val, exc_tb)
        if exc_type is None:
            split_excess_waits(self.nc)
        return ret


def r(ap):
    return ap.bitcast(F32R)


def _phase_k(nc, tc, pst, psm, kb, Wk, kpT, bks, ident):
    """kp^T projection -> SBUF f16 (Wk streamed in 1024-wide quarters)."""
    with (
        tc.tile_pool(name="kph", bufs=1) as kph,
        tc.tile_pool(name="wkq", bufs=2) as wkq,
        tc.tile_pool(name="kld", bufs=3) as kld,
    ):
        kT = kph.tile([P, HB, M], F32, tag="kT")
        wk_src = Wk.ap().rearrange("(hb p) d -> p hb d", p=P).bitcast(F32R)
        DQ = 512
        wk_first = wkq.tile([P, HB, DQ], F32, tag="wk")
        nc.sync.dma_start(out=wk_first.bitcast(F32R), in_=wk_src[:, :, 0:DQ])
        for mt in range(M // P):
            k_t = kld.tile([P, HIDDEN], F32, tag="ld")
            nc.sync.dma_start(out=k_t, in_=kb[mt * P : (mt + 1) * P, :])
            for hb in range(HB):
                pt = pst.tile([P, P], F32, tag="tp")
                nc.tensor.transpose(pt, k_t[:, hb * P : (hb + 1) * P], ident)
                nc.vector.tensor_copy(
                    kT[:, hb, mt * P : (mt + 1) * P].bitcast(F32R), pt.bitcast(F32R)
                )
        for dsl in range(PROJ // DQ):
            if dsl == 0:
                wk_t = wk_first
            else:
                wk_t = wkq.tile([P, HB, DQ], F32, tag="wk")
                nc.sync.dma_start(
                    out=wk_t.bitcast(F32R),
                    in_=wk_src[:, :, dsl * DQ : (dsl + 1) * DQ],
                )
            for db2 in range(DQ // P):
                db = dsl * (DQ // P) + db2
                for m2 in range(M // 512):
                    ps = psm.tile([P, 512], F32, tag="mm")
                    for hb in range(HB):
                        nc.tensor.matmul(
                            ps,
                            r(wk_t[:, hb, db2 * P : (db2 + 1) * P]),
                            r(kT[:, hb, m2 * 512 : (m2 + 1) * 512]),
                            start=(hb == 0),
                            stop=(hb == HB - 1),
                        )
                    nc.scalar.activation(
                        kpT[:, db, m2 * 512 : (m2 + 1) * 512],
                        ps,
                        AF.Identity,
                        bias=bks[:, db : db + 1],
                        scale=1.0,
                    )


def _emit_wq_cast(nc, tc, Wq, wq16):
    """Load Wq fp32 in 512-wide slices, cast to f16 on DVE (runs under K)."""
    with tc.tile_pool(name="wqld", bufs=2) as wqld:
        wq_src = Wq.ap().rearrange("(hb p) d -> p hb d", p=P).bitcast(F32R)
        W = 256
        for dsl in range(PROJ // W):
            wt = wqld.tile([P, HB, W], F32, tag="wq")
            nc.sync.dma_start(
                out=wt.bitcast(F32R), in_=wq_src[:, :, dsl * W : (dsl + 1) * W]
            )
            nc.vector.tensor_copy(wq16[:, :, dsl * W : (dsl + 1) * W], wt)


def _phase_ab(nc, tc, pst, psm, qb, vb, Wq, Wv, wq16, kpT, e, zp, bqs, vT, wv0,
              ident, eshift):
    """Fused q-projection + scores + exp per 512-wide n-chunk.

    The qp chunk buffer is single: PE program order is
    [qproj(i) | scores(i) | qproj(i+1) | ...], so by the time qproj(i+1)'s
    ACT drains overwrite qp[:, db, :], scores(i) has finished reading it.
    """
    with (
        tc.tile_pool(name="qpp", bufs=1) as qpp,
        tc.tile_pool(name="qld", bufs=2) as qld,
        tc.tile_pool(name="qTp", bufs=1) as qTp,
        tc.tile_pool(name="vld", bufs=2) as vld,
    ):
        def emit_qT(nb):
            qT = qTp.tile([P, HB, 512], F16, tag="qT")
            for nt in range(4):
                q_t = qld.tile([P, HIDDEN], F32, tag="ld")
                nc.sync.dma_start(
                    out=q_t, in_=qb[nb * 512 + nt * P : nb * 512 + (nt + 1) * P, :]
                )
                for hb in range(HB):
                    pt = pst.tile([P, P], F32, tag="tp")
                    nc.tensor.transpose(pt, q_t[:, hb * P : (hb + 1) * P], ident)
                    nc.vector.tensor_copy(
                        qT[:, hb, nt * P : (nt + 1) * P], pt
                    )
            return qT

        # Wq casts first in DVE program order so they drain during phase K
        # (qT copies would head-of-line-block DVE until K's matmuls finish).
        # The weight stream rides the ACT hwdge queue, q tiles ride SP.
        _emit_wq_cast(nc, tc, Wq, wq16)
        qT = emit_qT(0)
        qp = qpp.tile([P, DB, 512], F16)
        for nb in range(NB):
            # q-projection for this chunk: qp[:, db, :] = (qT @ wq16)*SCALE+bqs
            for db in range(DB):
                ps = psm.tile([P, 512], F32, tag="mm")
                for hb in range(HB):
                    nc.tensor.matmul(
                        ps,
                        wq16[:, hb, db * P : (db + 1) * P],
                        qT[:, hb, :],
                        start=(hb == 0),
                        stop=(hb == HB - 1),
                    )
                nc.scalar.activation(
                    qp[:, db, :], ps, AF.Identity,
                    bias=bqs[:, db : db + 1], scale=SCALE,
                )
            # next chunk's transposes (uses PE briefly; DMA has a full
            # scores window to land); the last two chunks pull v in for
            # phase V so its transposes hide under the scores matmuls
            if nb + 1 < NB:
                qT = emit_qT(nb + 1)
            if nb == 2:
                wv_src = Wv.ap().rearrange("(hb p) d -> p hb d", p=P).bitcast(F32R)
                nc.sync.dma_start(out=wv0.bitcast(F32R), in_=wv_src[:, :, 0:512])
            if nb >= 2:
                for mt in range(4):
                    mtt = (nb - 2) * 4 + mt
                    v_t = vld.tile([P, HIDDEN], F32, tag="ld")
                    nc.sync.dma_start(out=v_t, in_=vb[mtt * P : (mtt + 1) * P, :])
                    for hb in range(HB):
                        pt = pst.tile([P, P], F32, tag="tp")
                        nc.tensor.transpose(pt, v_t[:, hb * P : (hb + 1) * P], ident)
                        nc.vector.tensor_copy(
                            vT[:, hb, mtt * P : (mtt + 1) * P].bitcast(F32R),
                            pt.bitcast(F32R),
                        )
            # scores^T: e[:, mb, nchunk] = exp(kpT^T(d) @ qp - 40)
            for mb in range(MB):
                ps = psm.tile([P, 512], F32, tag="mm")
                for db in range(DB):
                    nc.tensor.matmul(
                        ps,
                        kpT[:, db, mb * P : (mb + 1) * P],
                        qp[:, db, :],
                        start=(db == 0),
                        stop=(db == DB - 1),
                    )
                nc.scalar.activation(
                    e[:, mb, nb * 512 : (nb + 1) * 512],
                    ps,
                    AF.Exp,
                    bias=eshift,
                    scale=1.0,
                    accum_out=zp[:, mb, nb : nb + 1],
                )


def _phase_v(nc, tc, pst, psm, Wv, bv, vT, vp, zp, Zt, rZ, wv0):
    """vp[:, mb, :] = (vT^T @ Wv + bv) * rZ[mb]  (bf16, SBUF-resident)."""
    with (
        tc.tile_pool(name="wvq", bufs=2) as wvq,
        tc.tile_pool(name="brow", bufs=1) as brow,
    ):
        for mb in range(MB):
            nc.vector.reduce_sum(Zt[:, mb : mb + 1], zp[:, mb, :], axis=AX)
        nc.vector.reciprocal(rZ, Zt)

        bvrow = brow.tile([1, PROJ], F32)
        nc.sync.dma_start(
            out=bvrow.bitcast(F32R),
            in_=bv.ap().rearrange("(o a) -> o a", o=1).bitcast(F32R),
        )
        ones_tmp = brow.tile([1, P], F32)
        nc.vector.memset(ones_tmp, 1.0)
        ones_row = brow.tile([1, P], F32)
        nc.vector.tensor_copy(ones_row.bitcast(F32R), ones_tmp.bitcast(F32R))

        wv_src = Wv.ap().rearrange("(hb p) d -> p hb d", p=P).bitcast(F32R)
        W = 512
        for dsl in range(PROJ // W):
            if dsl == 0:
                wv_t = wv0
            else:
                wv_t = wvq.tile([P, HB, W], F32, tag="wv")
                nc.sync.dma_start(
                    out=wv_t.bitcast(F32R), in_=wv_src[:, :, dsl * W : (dsl + 1) * W]
                )
            for ds2 in range(W // 512):
                dlo = dsl * W + ds2 * 512
                for mb in range(MB):
                    ps = psm.tile([P, 512], F32, tag="mm")
                    for hb in range(HB):
                        nc.tensor.matmul(
                            ps,
                            r(vT[:, hb, mb * P : (mb + 1) * P]),
                            r(wv_t[:, hb, ds2 * 512 : (ds2 + 1) * 512]),
                            start=(hb == 0),
                            stop=False,
                        )
                    nc.tensor.matmul(
                        ps,
                        r(ones_row),
                        r(bvrow[:, dlo : dlo + 512]),
                        start=False,
                        stop=True,
                    )
                    nc.scalar.activation(
                        vp[:, mb, dlo : dlo + 512],
                        ps,
                        AF.Identity,
                        scale=rZ[:, mb : mb + 1],
                    )


def _phase_c(nc, tc, psm, Wo, vp, e, wo16, outT):
    """x^T = vp^T @ e per d-block, out^T = Wo^T @ x^T -> DRAM.

    Like qp in phase AB, x_s is a single buffer: PE order is
    [x(nb) | out(nb) | x(nb+1) | ...] so out(nb) has finished reading
    x_s before x(nb+1)'s drains overwrite it. The Wo f32->f16 cast is
    emitted first; its DMA + DVE copies hide under the first x block.
    """
    with (
        tc.tile_pool(name="wold", bufs=2) as wold,
        tc.tile_pool(name="xsp", bufs=1) as xsp,
        tc.tile_pool(name="osp", bufs=2) as osp,
    ):
        wo_src = Wo.ap().rearrange("(db p) h -> p db h", p=P).bitcast(F32R)
        for wsl in range(8):
            wo_t = wold.tile([P, 4, HIDDEN], F32, tag="wo")
            nc.sync.dma_start(
                out=wo_t.bitcast(F32R), in_=wo_src[:, wsl * 4 : (wsl + 1) * 4, :]
            )
            nc.vector.tensor_copy(wo16[:, wsl * 4 : (wsl + 1) * 4, :], wo_t)

        x_s = xsp.tile([P, DB, 512], F16, tag="x")
        for nb in range(NB):
            for db in range(DB):
                ps = psm.tile([P, 512], F32, tag="mm")
                for mch in range(MB):
                    nc.tensor.matmul(
                        ps,
                        vp[:, mch, db * P : (db + 1) * P],
                        e[:, mch, nb * 512 : (nb + 1) * 512],
                        start=(mch == 0),
                        stop=(mch == MB - 1),
                    )
                nc.vector.tensor_copy(x_s[:, db, :], ps)
            for hb in range(HB):
                if nb == NB - 1 and hb == HB - 1:
                    # split the final chain so its drain+store pipelines
                    # instead of sitting fully after the last matmul
                    for nh in range(2):
                        ps2 = psm.tile([P, 512], F32, tag="mm")
                        for db in range(DB):
                            nc.tensor.matmul(
                                ps2[:, 0:256],
                                wo16[:, db, hb * P : (hb + 1) * P],
                                x_s[:, db, nh * 256 : (nh + 1) * 256],
                                start=(db == 0),
                                stop=(db == DB - 1),
                            )
                        ot = osp.tile([P, 512], F32, tag="ot")
                        nc.vector.tensor_copy(ot[:, 0:256], ps2[:, 0:256])
                        nc.sync.dma_start(
                            out=outT[
                                hb * P : (hb + 1) * P,
                                nb * 512 + nh * 256 : nb * 512 + (nh + 1) * 256,
                            ],
                            in_=ot[:, 0:256],
                        )
                    continue
                ps2 = psm.tile([P, 512], F32, tag="mm")
                for db in range(DB):
                    nc.tensor.matmul(
                        ps2,
                        wo16[:, db, hb * P : (hb + 1) * P],
                        x_s[:, db, :],
                        start=(db == 0),
                        stop=(db == DB - 1),
                    )
                ot = osp.tile([P, 512], F32, tag="ot")
                nc.vector.tensor_copy(ot, ps2)
                nc.sync.dma_start(
                    out=outT[hb * P : (hb + 1) * P, nb * 512 : (nb + 1) * 512],
                    in_=ot,
                )


def build_nc():
    nc = bass.Bass("TRN2", target_bir_lowering=False, debug=False, num_devices=8)

    qb = nc.dram_tensor("qb", [N, HIDDEN], F32, kind="ExternalInput")
    kb = nc.dram_tensor("kb", [M, HIDDEN], F32, kind="ExternalInput")
    vb = nc.dram_tensor("vb", [M, HIDDEN], F32, kind="ExternalInput")
    Wq = nc.dram_tensor("Wq", [HIDDEN, PROJ], F32, kind="ExternalInput")
    Wk = nc.dram_tensor("Wk", [HIDDEN, PROJ], F32, kind="ExternalInput")
    Wv = nc.dram_tensor("Wv", [HIDDEN, PROJ], F32, kind="ExternalInput")
    Wo = nc.dram_tensor("Wo", [PROJ, HIDDEN], F32, kind="ExternalInput")
    bq = nc.dram_tensor("bq", [PROJ], F32, kind="ExternalInput")
    bk = nc.dram_tensor("bk", [PROJ], F32, kind="ExternalInput")
    bv = nc.dram_tensor("bv", [PROJ], F32, kind="ExternalInput")
    outT = nc.dram_tensor("outT", [HIDDEN, N], F32, kind="ExternalOutput")

    with PatchedTC(nc) as tc:
        with (
            tc.tile_pool(name="singles", bufs=1) as singles,
            tc.tile_pool(name="pst", bufs=2, space="PSUM") as pst,
            tc.tile_pool(name="psm", bufs=6, space="PSUM") as psm,
        ):
            ident = singles.tile([P, P], F32)
            make_identity(nc, ident)
            # biases need (p, db) layout with d inner on partitions; a direct
            # strided DMA would be 4096 4-byte descriptors, so load the
            # contiguous [DB, P] view and PE-transpose it instead.
            bq_raw = singles.tile([DB, P], F32)
            nc.sync.dma_start(out=bq_raw, in_=bq.ap().rearrange("(a b) -> a b", b=P))
            bqs = singles.tile([P, DB], F32)
            ptb = pst.tile([P, DB], F32, tag="tp")
            nc.tensor.transpose(ptb, bq_raw, ident[:DB, :DB])
            nc.scalar.activation(bqs, ptb, AF.Identity, scale=SCALE)
            bk_raw = singles.tile([DB, P], F32)
            nc.sync.dma_start(out=bk_raw, in_=bk.ap().rearrange("(a b) -> a b", b=P))
            bks = singles.tile([P, DB], F32)
            ptb2 = pst.tile([P, DB], F32, tag="tp")
            nc.tensor.transpose(ptb2, bk_raw, ident[:DB, :DB])
            nc.vector.tensor_copy(bks, ptb2)
            zp = singles.tile([P, MB, NB], F32)
            Zt = singles.tile([P, MB], F32)
            rZ = singles.tile([P, MB], F32)
            eshift = singles.tile([P, 1], F32)
            nc.vector.memset(eshift, EXP_SHIFT)

            # e (bf16 scores-exp) lives from AB through C.
            with tc.tile_pool(name="e_lvl", bufs=1) as e_lvl:
                e = e_lvl.tile([P, MB, N], BF16)
                vT = e_lvl.tile([P, HB, M], F32)
                wv0 = e_lvl.tile([P, HB, 512], F32)
                with tc.tile_pool(name="ab", bufs=1) as ab:
                    kpT = ab.tile([P, DB, M], F16)
                    wq16 = ab.tile([P, HB, PROJ], F16)
                    _phase_k(nc, tc, pst, psm, kb, Wk, kpT, bks, ident)
                    _phase_ab(
                        nc, tc, pst, psm, qb, vb, Wq, Wv, wq16, kpT, e, zp, bqs,
                        vT, wv0, ident, eshift,
                    )
                with tc.tile_pool(name="vc", bufs=1) as vc:
                    vp = vc.tile([P, MB, PROJ], BF16)
                    _phase_v(nc, tc, pst, psm, Wv, bv, vT, vp, zp, Zt, rZ, wv0)
                    wo16 = vc.tile([P, DB, HIDDEN], F16)
                    _phase_c(nc, tc, psm, Wo, vp, e, wo16, outT)
    # A handful of waits are attached after the TileContext's own exit
    # processing; sweep again until the module is clean.
    while split_excess_waits(nc):
        pass
    return nc


class _Runner:
    """Compile the Bass program once; re-execute cheaply on later calls.

    Mirrors bass2jax.run_bass_via_pjrt's multi-core path, but keeps the
    jitted shard_map callable so repeated kernel() calls skip the
    multi-minute neuronxcc compile.
    """

    def __init__(self):
        import jax
        from jax.sharding import Mesh, PartitionSpec
        from jax.experimental.shard_map import shard_map
        from concourse import bass2jax
        import concourse.mybir as mb

        self.jax = jax
        nc = build_nc()
        self.nc = nc
        bass2jax.install_neuronx_cc_hook()

        in_names, out_names, out_avals, zero_outs = [], [], [], []
        partition_name = (
            nc.partition_id_tensor.name if nc.partition_id_tensor else None
        )
        for alloc in nc.m.functions[0].allocations:
            if not isinstance(alloc, mb.MemoryLocationSet):
                continue
            name = alloc.memorylocations[0].name
            if alloc.kind == "ExternalInput":
                if name != partition_name:
                    in_names.append(name)
            elif alloc.kind == "ExternalOutput":
                shape = tuple(alloc.tensor_shape)
                dtype = mb.dt.np(alloc.dtype)
                out_names.append(name)
                out_avals.append(jax.core.ShapedArray(shape, dtype))
                zero_outs.append(np.zeros(shape, dtype))
        n_params = len(in_names)
        n_outs = len(out_avals)
        all_in_names = list(in_names) + list(out_names)
        if partition_name is not None:
            all_in_names.append(partition_name)
        self.in_names = in_names
        self.out_names = out_names
        self.zero_outs = zero_outs

        def _body(*args):
            operands = list(args)
            if partition_name is not None:
                operands.append(bass2jax.partition_id_tensor())
            outs = bass2jax._bass_exec_p.bind(
                *operands,
                out_avals=tuple(out_avals),
                in_names=tuple(all_in_names),
                out_names=tuple(out_names),
                lowering_input_output_aliases=(),
                sim_require_finite=True,
                sim_require_nnan=True,
                nc=nc,
            )
            return tuple(outs)

        devices = jax.devices()[:8]
        mesh = Mesh(np.asarray(devices), ("core",))
        self.mesh = mesh
        in_specs = (PartitionSpec("core"),) * (n_params + n_outs)
        out_specs = (PartitionSpec("core"),) * n_outs
        self.body = _body
        self.in_specs = in_specs
        self.out_specs = out_specs
        donate = tuple(range(n_params, n_params + n_outs))
        self.sharded = jax.jit(
            shard_map(
                _body,
                mesh=mesh,
                in_specs=in_specs,
                out_specs=out_specs,
                check_rep=False,
            ),
            donate_argnums=donate,
            keep_unused=True,
        )
        self.out_avals = out_avals

    def prepare(self, in_maps):
        """Concatenate per-core inputs along axis 0 (device-shardable)."""
        return [
            np.concatenate([in_maps[c][name] for c in range(8)], axis=0)
            for name in self.in_names
        ]

    def run(self, concat_in):
        zeros = [
            np.zeros((8 * z.shape[0], *z.shape[1:]), z.dtype) for z in self.zero_outs
        ]
        out_arrs = self.sharded(*concat_in, *zeros)
        res = []
        for c in range(8):
            res.append(
                {
                    name: np.asarray(out_arrs[i]).reshape(
                        8, *self.out_avals[i].shape
                    )[c]
                    for i, name in enumerate(self.out_names)
                }
            )
        return res


_RUNNER = None


def _get_runner():
    global _RUNNER
    if _RUNNER is None:
        _RUNNER = _Runner()
    return _RUNNER


def make_in_maps(inputs):
    f32 = lambda x: np.ascontiguousarray(np.asarray(x, dtype=np.float32))
    q, k, v = f32(inputs["q"]), f32(inputs["k"]), f32(inputs["v"])
    Wq, Wk, Wv, Wo = (f32(inputs[n]) for n in ("Wq", "Wk", "Wv", "Wo"))
    bq, bk, bv = (f32(inputs[n]) for n in ("bq", "bk", "bv"))
    in_maps = []
    for c in range(8):
        b, mh = c // 2, c % 2
        sl = slice(mh * M, (mh + 1) * M)
        in_maps.append(
            {
                "qb": q[b],
                "kb": np.ascontiguousarray(k[b, sl]),
                "vb": np.ascontiguousarray(v[b, sl]),
                "Wq": Wq, "Wk": Wk, "Wv": Wv, "Wo": Wo,
                "bq": bq, "bk": bk, "bv": bv,
            }
        )
    return in_maps


def assemble_out(results, bo):
    out = np.empty((B, N, HIDDEN), dtype=np.float32)
    for b in range(B):
        acc = results[2 * b]["outT"] + results[2 * b + 1]["outT"]
        out[b] = acc.T + bo[None, :]
    return out


def kernel(**inputs):
    runner = _get_runner()
    res = runner.run(runner.prepare(make_in_maps(inputs)))
    bo = np.asarray(inputs["bo"], dtype=np.float32)
    return assemble_out(res, bo)


# revision 25
# speedup vs baseline: 2.2465x; 1.1100x over previous
"""Trainium2 Bass kernel for nn_MultiHeadAttention_79706003079680.

Reference (fp32):
    qp = (q @ Wq + bq) * SCALE      # [B, N, PROJ]
    kp = k @ Wk + bk
    vp = v @ Wv + bv
    scores = einsum('bnd,bmd->bnm', qp, kp)
    attn = softmax(scores, axis=1)          # over the QUERY axis n
    x = einsum('bnm,bmd->bnd', attn, vp)
    out = x @ Wo + bo                       # [B, N, HIDDEN]

Sharding: 8 cores = 4 batches x 2 key-halves (m in [mh*1024, mh*1024+1024)).
Softmax over n couples all queries for a fixed key m, so each core keeps
all n=2048 queries and a slice of keys. Each core emits a partial
out^T [HIDDEN, N]; the host sums the two key-halves per batch, transposes,
and adds bo.

Single-pass structure (everything SBUF-resident, no DRAM round-trips):
  K:  kp^T [P, DB, M] f16 resident (Wk fp32r, bias via ACT drain).
  AB: per 512-wide n-chunk: project q chunk (wq16 f16 resident, qp chunk
      f16 in SBUF only) then scores^T for all 8 m-blocks at N=512 free dim.
      exp() is applied directly on the PSUM drain with a constant -40 bias
      (softmax normalizer is deferred: e' = exp(s-40) and Z' = sum_n e'
      cancel in e'/Z', so no per-column max pass is needed).
  V:  vp = (v @ Wv + bv) * (1/Z') folded into the ACT drain (scale=rZ AP),
      bf16 resident.
  C:  x^T = vp^T @ e per d-block (f16), out^T = Wo^T @ x^T, DMA to DRAM.

All big matmuls run at 1 PE cycle/row and 512-wide moving operands:
float32r (fp32 truncated to FP22) for the k/v projections, f16/bf16 for
q-projection / scores / x / out.
"""

import numpy as np

import concourse.bass as bass
import concourse.mybir as mybir
import concourse.tile as tile
from concourse.masks import make_identity

P = 128
HIDDEN = 512
NUM_HEADS = 8
PROJ = NUM_HEADS * HIDDEN          # 4096
B, N = 4, 2048
M = N // 2                         # keys per core = 1024
SCALE = (HIDDEN // NUM_HEADS) ** -0.5

HB = HIDDEN // P                   # 4 h-blocks of 128
DB = PROJ // P                     # 32 d-blocks of 128
NB = N // 512                      # 4 n-chunks of 512
MB = M // P                        # 8 m-blocks of 128
EXP_SHIFT = -40.0                  # constant exp bias; cancels in e/Z

F32 = mybir.dt.float32
F32R = mybir.dt.float32r
F16 = mybir.dt.float16
BF16 = mybir.dt.bfloat16
AX = mybir.AxisListType.X
AF = mybir.ActivationFunctionType


MAX_WAITS = 1


def split_excess_waits(nc, max_waits=MAX_WAITS):
    """Move excess per-instruction sem waits onto same-engine NoOps.

    This walrus build rejects instructions carrying more than a couple of
    sync-wait commands ("Too many sync wait commands" in setupSyncWait).
    A NoOp placed immediately before the instruction on the same engine
    enforces the wait in program order with identical semantics.
    """
    n_extra = 0
    for f in nc.m.functions:
        for bb in f.blocks:
            insts = bb.instructions
            i = 0
            while i < len(insts):
                inst = insts[i]
                si = getattr(inst, "sync_info", None)
                if si is not None and si.on_wait and len(si.on_wait) > max_waits:
                    waits = list(si.on_wait)
                    si.on_wait = waits[: max_waits]
                    for w in waits[max_waits:]:
                        n_extra += 1
                        nop = mybir.InstNoOp(
                            name=f"I-wsplit{n_extra}",
                            ins=[],
                            outs=[],
                            engine=inst.engine,
                        )
                        nop.sync_info = mybir.SyncInfo(on_wait=[w], on_update=[])
                        try:
                            nc.register_instruction(nop)
                        except Exception:
                            pass
                        # insert immediately before inst (inst shifts right)
                        insts.insert(i, nop)
                        i += 1
                i += 1
    return n_extra


class PatchedTC(tile.TileContext):
    """TileContext that post-processes the module to satisfy this walrus
    build's per-instruction sync-wait limit."""

    def __exit__(self, exc_type, exc_val, exc_tb):
        ret = super().__exit__(exc_type, exc_val, exc_tb)
        if exc_type is None:
            split_excess_waits(self.nc)
        return ret


def r(ap):
    return ap.bitcast(F32R)


def _phase_k(nc, tc, pst, psm, kb, Wk, kpT, bks, ident):
    """kp^T projection -> SBUF f16 (Wk streamed in 512-wide slices; slice 0
    is issued before the k loads so the first matmul chain starts early)."""
    with (
        tc.tile_pool(name="kph", bufs=1) as kph,
        tc.tile_pool(name="wkq", bufs=2) as wkq,
        tc.tile_pool(name="kld", bufs=3) as kld,
    ):
        kT = kph.tile([P, HB, M], F32, tag="kT")
        wk_src = Wk.ap().rearrange("(hb p) d -> p hb d", p=P).bitcast(F32R)
        DQ = 512
        wk_first = wkq.tile([P, HB, DQ], F32, tag="wk")
        nc.sync.dma_start(out=wk_first.bitcast(F32R), in_=wk_src[:, :, 0:DQ])
        for mt in range(M // P):
            k_t = kld.tile([P, HIDDEN], F32, tag="ld")
            nc.sync.dma_start(out=k_t, in_=kb[mt * P : (mt + 1) * P, :])
            for hb in range(HB):
                pt = pst.tile([P, P], F32, tag="tp")
                nc.tensor.transpose(pt, k_t[:, hb * P : (hb + 1) * P], ident)
                nc.vector.tensor_copy(
                    kT[:, hb, mt * P : (mt + 1) * P].bitcast(F32R), pt.bitcast(F32R)
                )
        for dsl in range(PROJ // DQ):
            if dsl == 0:
                wk_t = wk_first
            else:
                wk_t = wkq.tile([P, HB, DQ], F32, tag="wk")
                nc.sync.dma_start(
                    out=wk_t.bitcast(F32R),
                    in_=wk_src[:, :, dsl * DQ : (dsl + 1) * DQ],
                )
            for db2 in range(DQ // P):
                db = dsl * (DQ // P) + db2
                for m2 in range(M // 512):
                    ps = psm.tile([P, 512], F32, tag="mm")
                    for hb in range(HB):
                        nc.tensor.matmul(
                            ps,
                            r(wk_t[:, hb, db2 * P : (db2 + 1) * P]),
                            r(kT[:, hb, m2 * 512 : (m2 + 1) * 512]),
                            start=(hb == 0),
                            stop=(hb == HB - 1),
                        )
                    nc.scalar.activation(
                        kpT[:, db, m2 * 512 : (m2 + 1) * 512],
                        ps,
                        AF.Identity,
                        bias=bks[:, db : db + 1],
                        scale=1.0,
                    )


def _emit_wq_cast(nc, tc, Wq, wq16):
    """Load Wq fp32 in 512-wide slices, cast to f16 on DVE (runs under K)."""
    with tc.tile_pool(name="wqld", bufs=2) as wqld:
        wq_src = Wq.ap().rearrange("(hb p) d -> p hb d", p=P).bitcast(F32R)
        W = 256
        for dsl in range(PROJ // W):
            wt = wqld.tile([P, HB, W], F32, tag="wq")
            nc.sync.dma_start(
                out=wt.bitcast(F32R), in_=wq_src[:, :, dsl * W : (dsl + 1) * W]
            )
            nc.vector.tensor_copy(wq16[:, :, dsl * W : (dsl + 1) * W], wt)


def _phase_ab(nc, tc, pst, psm, qb, vb, Wq, Wv, wq16, kpT, e, zp, bqs, vT, wv0,
              ident, eshift):
    """Fused q-projection + scores + exp per 512-wide n-chunk.

    The qp chunk buffer is single: PE program order is
    [qproj(i) | scores(i) | qproj(i+1) | ...], so by the time qproj(i+1)'s
    ACT drains overwrite qp[:, db, :], scores(i) has finished reading it.
    """
    with (
        tc.tile_pool(name="qpp", bufs=1) as qpp,
        tc.tile_pool(name="qld", bufs=2) as qld,
        tc.tile_pool(name="qTp", bufs=1) as qTp,
        tc.tile_pool(name="vld", bufs=2) as vld,
    ):
        def emit_qT(nb):
            qT = qTp.tile([P, HB, 512], F16, tag="qT")
            for nt in range(4):
                q_t = qld.tile([P, HIDDEN], F32, tag="ld")
                nc.sync.dma_start(
                    out=q_t, in_=qb[nb * 512 + nt * P : nb * 512 + (nt + 1) * P, :]
                )
                for hb in range(HB):
                    pt = pst.tile([P, P], F32, tag="tp")
                    nc.tensor.transpose(pt, q_t[:, hb * P : (hb + 1) * P], ident)
                    nc.vector.tensor_copy(
                        qT[:, hb, nt * P : (nt + 1) * P], pt
                    )
            return qT

        # Wq casts first in DVE program order so they drain during phase K
        # (qT copies would head-of-line-block DVE until K's matmuls finish).
        # The weight stream rides the ACT hwdge queue, q tiles ride SP.
        _emit_wq_cast(nc, tc, Wq, wq16)
        qT = emit_qT(0)
        qp = qpp.tile([P, DB, 512], F16)
        for nb in range(NB):
            # q-projection for this chunk: qp[:, db, :] = (qT @ wq16)*SCALE+bqs
            for db in range(DB):
                ps = psm.tile([P, 512], F32, tag="mm")
                for hb in range(HB):
                    nc.tensor.matmul(
                        ps,
                        wq16[:, hb, db * P : (db + 1) * P],
                        qT[:, hb, :],
                        start=(hb == 0),
                        stop=(hb == HB - 1),
                    )
                nc.scalar.activation(
                    qp[:, db, :], ps, AF.Identity,
                    bias=bqs[:, db : db + 1], scale=SCALE,
                )
            # next chunk's transposes (uses PE briefly; DMA has a full
            # scores window to land); the last two chunks pull v in for
            # phase V so its transposes hide under the scores matmuls
            if nb + 1 < NB:
                qT = emit_qT(nb + 1)
            if nb == 2:
                wv_src = Wv.ap().rearrange("(hb p) d -> p hb d", p=P).bitcast(F32R)
                nc.sync.dma_start(out=wv0.bitcast(F32R), in_=wv_src[:, :, 0:512])
            if nb >= 2:
                for mt in range(4):
                    mtt = (nb - 2) * 4 + mt
                    v_t = vld.tile([P, HIDDEN], F32, tag="ld")
                    nc.sync.dma_start(out=v_t, in_=vb[mtt * P : (mtt + 1) * P, :])
                    for hb in range(HB):
                        pt = pst.tile([P, P], F32, tag="tp")
                        nc.tensor.transpose(pt, v_t[:, hb * P : (hb + 1) * P], ident)
                        nc.vector.tensor_copy(
                            vT[:, hb, mtt * P : (mtt + 1) * P].bitcast(F32R),
                            pt.bitcast(F32R),
                        )
            # scores^T: e[:, mb, nchunk] = exp(kpT^T(d) @ qp - 40)
            for mb in range(MB):
                ps = psm.tile([P, 512], F32, tag="mm")
                for db in range(DB):
                    nc.tensor.matmul(
                        ps,
                        kpT[:, db, mb * P : (mb + 1) * P],
                        qp[:, db, :],
                        start=(db == 0),
                        stop=(db == DB - 1),
                    )
                nc.scalar.activation(
                    e[:, mb, nb * 512 : (nb + 1) * 512],
                    ps,
                    AF.Exp,
                    bias=eshift,
                    scale=1.0,
                    accum_out=zp[:, mb, nb : nb + 1],
                )


def _phase_v(nc, tc, pst, psm, Wv, bv, vT, vp, zp, Zt, rZ, wv0):
    """vp[:, mb, :] = (vT^T @ Wv + bv) * rZ[mb]  (bf16, SBUF-resident)."""
    with (
        tc.tile_pool(name="wvq", bufs=2) as wvq,
        tc.tile_pool(name="brow", bufs=1) as brow,
    ):
        for mb in range(MB):
            nc.vector.reduce_sum(Zt[:, mb : mb + 1], zp[:, mb, :], axis=AX)
        nc.vector.reciprocal(rZ, Zt)

        bvrow = brow.tile([1, PROJ], F32)
        nc.sync.dma_start(
            out=bvrow.bitcast(F32R),
            in_=bv.ap().rearrange("(o a) -> o a", o=1).bitcast(F32R),
        )
        ones_tmp = brow.tile([1, P], F32)
        nc.vector.memset(ones_tmp, 1.0)
        ones_row = brow.tile([1, P], F32)
        nc.vector.tensor_copy(ones_row.bitcast(F32R), ones_tmp.bitcast(F32R))

        wv_src = Wv.ap().rearrange("(hb p) d -> p hb d", p=P).bitcast(F32R)
        W = 512
        for dsl in range(PROJ // W):
            if dsl == 0:
                wv_t = wv0
            else:
                wv_t = wvq.tile([P, HB, W], F32, tag="wv")
                nc.sync.dma_start(
                    out=wv_t.bitcast(F32R), in_=wv_src[:, :, dsl * W : (dsl + 1) * W]
                )
            for ds2 in range(W // 512):
                dlo = dsl * W + ds2 * 512
                for mb in range(MB):
                    ps = psm.tile([P, 512], F32, tag="mm")
                    for hb in range(HB):
                        nc.tensor.matmul(
                            ps,
                            r(vT[:, hb, mb * P : (mb + 1) * P]),
                            r(wv_t[:, hb, ds2 * 512 : (ds2 + 1) * 512]),
                            start=(hb == 0),
                            stop=False,
                        )
                    nc.tensor.matmul(
                        ps,
                        r(ones_row),
                        r(bvrow[:, dlo : dlo + 512]),
                        start=False,
                        stop=True,
                    )
                    nc.scalar.activation(
                        vp[:, mb, dlo : dlo + 512],
                        ps,
                        AF.Identity,
                        scale=rZ[:, mb : mb + 1],
                    )


def _phase_c(nc, tc, psm, Wo, vp, e, wo16, outT):
    """x^T = vp^T @ e per d-block, out^T = Wo^T @ x^T -> DRAM.

    Like qp in phase AB, x_s is a single buffer: PE order is
    [x(nb) | out(nb) | x(nb+1) | ...] so out(nb) has finished reading
    x_s before x(nb+1)'s drains overwrite it. The Wo f32->f16 cast is
    emitted first; its DMA + DVE copies hide under the first x block.
    """
    with (
        tc.tile_pool(name="wold", bufs=2) as wold,
        tc.tile_pool(name="xsp", bufs=1) as xsp,
        tc.tile_pool(name="osp", bufs=2) as osp,
    ):
        wo_src = Wo.ap().rearrange("(db p) h -> p db h", p=P).bitcast(F32R)
        for wsl in range(8):
            wo_t = wold.tile([P, 4, HIDDEN], F32, tag="wo")
            nc.sync.dma_start(
                out=wo_t.bitcast(F32R), in_=wo_src[:, wsl * 4 : (wsl + 1) * 4, :]
            )
            nc.vector.tensor_copy(wo16[:, wsl * 4 : (wsl + 1) * 4, :], wo_t)

        x_s = xsp.tile([P, DB, 512], F16, tag="x")
        for nb in range(NB):
            for db in range(DB):
                ps = psm.tile([P, 512], F32, tag="mm")
                for mch in range(MB):
                    nc.tensor.matmul(
                        ps,
                        vp[:, mch, db * P : (db + 1) * P],
                        e[:, mch, nb * 512 : (nb + 1) * 512],
                        start=(mch == 0),
                        stop=(mch == MB - 1),
                    )
                nc.vector.tensor_copy(x_s[:, db, :], ps)
            for hb in range(HB):
                if nb == NB - 1 and hb == HB - 1:
                    # split the final chain so its drain+store pipelines
                    # instead of sitting fully after the last matmul
                    for nh in range(2):
                        ps2 = psm.tile([P, 512], F32, tag="mm")
                        for db in range(DB):
                            nc.tensor.matmul(
                                ps2[:, 0:256],
                                wo16[:, db, hb * P : (hb + 1) * P],
                                x_s[:, db, nh * 256 : (nh + 1) * 256],
                                start=(db == 0),
                                stop=(db == DB - 1),
                            )
                        ot = osp.tile([P, 512], F32, tag="ot")
                        nc.vector.tensor_copy(ot[:, 0:256], ps2[:, 0:256])
                        nc.sync.dma_start(
                            out=outT[
                                hb * P : (hb + 1) * P,
                                nb * 512 + nh * 256 : nb * 512 + (nh + 1) * 256,
                            ],
                            in_=ot[:, 0:256],
                        )
                    continue
                ps2 = psm.tile([P, 512], F32, tag="mm")
                for db in range(DB):
                    nc.tensor.matmul(
                        ps2,
                        wo16[:, db, hb * P : (hb + 1) * P],
                        x_s[:, db, :],
                        start=(db == 0),
                        stop=(db == DB - 1),
                    )
                ot = osp.tile([P, 512], F32, tag="ot")
                nc.vector.tensor_copy(ot, ps2)
                nc.sync.dma_start(
                    out=outT[hb * P : (hb + 1) * P, nb * 512 : (nb + 1) * 512],
                    in_=ot,
                )


def build_nc():
    nc = bass.Bass("TRN2", target_bir_lowering=False, debug=False, num_devices=8)

    qb = nc.dram_tensor("qb", [N, HIDDEN], F32, kind="ExternalInput")
    kb = nc.dram_tensor("kb", [M, HIDDEN], F32, kind="ExternalInput")
    vb = nc.dram_tensor("vb", [M, HIDDEN], F32, kind="ExternalInput")
    Wq = nc.dram_tensor("Wq", [HIDDEN, PROJ], F32, kind="ExternalInput")
    Wk = nc.dram_tensor("Wk", [HIDDEN, PROJ], F32, kind="ExternalInput")
    Wv = nc.dram_tensor("Wv", [HIDDEN, PROJ], F32, kind="ExternalInput")
    Wo = nc.dram_tensor("Wo", [PROJ, HIDDEN], F32, kind="ExternalInput")
    bq = nc.dram_tensor("bq", [PROJ], F32, kind="ExternalInput")
    bk = nc.dram_tensor("bk", [PROJ], F32, kind="ExternalInput")
    bv = nc.dram_tensor("bv", [PROJ], F32, kind="ExternalInput")
    outT = nc.dram_tensor("outT", [HIDDEN, N], F32, kind="ExternalOutput")

    with PatchedTC(nc) as tc:
        with (
            tc.tile_pool(name="singles", bufs=1) as singles,
            tc.tile_pool(name="pst", bufs=2, space="PSUM") as pst,
            tc.tile_pool(name="psm", bufs=6, space="PSUM") as psm,
        ):
            ident = singles.tile([P, P], F32)
            make_identity(nc, ident)
            # biases need (p, db) layout with d inner on partitions; a direct
            # strided DMA would be 4096 4-byte descriptors, so load the
            # contiguous [DB, P] view and PE-transpose it instead.
            bq_raw = singles.tile([DB, P], F32)
            nc.sync.dma_start(out=bq_raw, in_=bq.ap().rearrange("(a b) -> a b", b=P))
            bqs = singles.tile([P, DB], F32)
            ptb = pst.tile([P, DB], F32, tag="tp")
            nc.tensor.transpose(ptb, bq_raw, ident[:DB, :DB])
            nc.scalar.activation(bqs, ptb, AF.Identity, scale=SCALE)
            bk_raw = singles.tile([DB, P], F32)
            nc.sync.dma_start(out=bk_raw, in_=bk.ap().rearrange("(a b) -> a b", b=P))
            bks = singles.tile([P, DB], F32)
            ptb2 = pst.tile([P, DB], F32, tag="tp")
            nc.tensor.transpose(ptb2, bk_raw, ident[:DB, :DB])
            nc.vector.tensor_copy(bks, ptb2)
            zp = singles.tile([P, MB, NB], F32)
            Zt = singles.tile([P, MB], F32)
            rZ = singles.tile([P, MB], F32)
            eshift = singles.tile([P, 1], F32)
            nc.vector.memset(eshift, EXP_SHIFT)

            # e (bf16 scores-exp) lives from AB through C.
            with tc.tile_pool(name="e_lvl", bufs=1) as e_lvl:
                e = e_lvl.tile([P, MB, N], BF16)
                vT = e_lvl.tile([P, HB, M], F32)
                wv0 = e_lvl.tile([P, HB, 512], F32)
                with tc.tile_pool(name="ab", bufs=1) as ab:
                    kpT = ab.tile([P, DB, M], F16)
                    wq16 = ab.tile([P, HB, PROJ], F16)
                    _phase_k(nc, tc, pst, psm, kb, Wk, kpT, bks, ident)
                    _phase_ab(
                        nc, tc, pst, psm, qb, vb, Wq, Wv, wq16, kpT, e, zp, bqs,
                        vT, wv0, ident, eshift,
                    )
                with tc.tile_pool(name="vc", bufs=1) as vc:
                    vp = vc.tile([P, MB, PROJ], BF16)
                    _phase_v(nc, tc, pst, psm, Wv, bv, vT, vp, zp, Zt, rZ, wv0)
                    wo16 = vc.tile([P, DB, HIDDEN], F16)
                    _phase_c(nc, tc, psm, Wo, vp, e, wo16, outT)
    # A handful of waits are attached after the TileContext's own exit
    # processing; sweep again until the module is clean.
    while split_excess_waits(nc):
        pass
    return nc


class _Runner:
    """Compile the Bass program once; re-execute cheaply on later calls.

    Mirrors bass2jax.run_bass_via_pjrt's multi-core path, but keeps the
    jitted shard_map callable so repeated kernel() calls skip the
    multi-minute neuronxcc compile.
    """

    def __init__(self):
        import jax
        from jax.sharding import Mesh, PartitionSpec
        from jax.experimental.shard_map import shard_map
        from concourse import bass2jax
        import concourse.mybir as mb

        self.jax = jax
        nc = build_nc()
        self.nc = nc
        bass2jax.install_neuronx_cc_hook()

        in_names, out_names, out_avals, zero_outs = [], [], [], []
        partition_name = (
            nc.partition_id_tensor.name if nc.partition_id_tensor else None
        )
        for alloc in nc.m.functions[0].allocations:
            if not isinstance(alloc, mb.MemoryLocationSet):
                continue
            name = alloc.memorylocations[0].name
            if alloc.kind == "ExternalInput":
                if name != partition_name:
                    in_names.append(name)
            elif alloc.kind == "ExternalOutput":
                shape = tuple(alloc.tensor_shape)
                dtype = mb.dt.np(alloc.dtype)
                out_names.append(name)
                out_avals.append(jax.core.ShapedArray(shape, dtype))
                zero_outs.append(np.zeros(shape, dtype))
        n_params = len(in_names)
        n_outs = len(out_avals)
        all_in_names = list(in_names) + list(out_names)
        if partition_name is not None:
            all_in_names.append(partition_name)
        self.in_names = in_names
        self.out_names = out_names
        self.zero_outs = zero_outs

        def _body(*args):
            operands = list(args)
            if partition_name is not None:
                operands.append(bass2jax.partition_id_tensor())
            outs = bass2jax._bass_exec_p.bind(
                *operands,
                out_avals=tuple(out_avals),
                in_names=tuple(all_in_names),
                out_names=tuple(out_names),
                lowering_input_output_aliases=(),
                sim_require_finite=True,
                sim_require_nnan=True,
                nc=nc,
            )
            return tuple(outs)

        devices = jax.devices()[:8]
        mesh = Mesh(np.asarray(devices), ("core",))
        self.mesh = mesh
        in_specs = (PartitionSpec("core"),) * (n_params + n_outs)
        out_specs = (PartitionSpec("core"),) * n_outs
        self.body = _body
        self.in_specs = in_specs
        self.out_specs = out_specs
        donate = tuple(range(n_params, n_params + n_outs))
        self.sharded = jax.jit(
            shard_map(
                _body,
                mesh=mesh,
                in_specs=in_specs,
                out_specs=out_specs,
                check_rep=False,
            ),
            donate_argnums=donate,
            keep_unused=True,
        )
        self.out_avals = out_avals

    def prepare(self, in_maps):
        """Concatenate per-core inputs along axis 0 (device-shardable)."""
        return [
            np.concatenate([in_maps[c][name] for c in range(8)], axis=0)
            for name in self.in_names
        ]

    def run(self, concat_in):
        zeros = [
            np.zeros((8 * z.shape[0], *z.shape[1:]), z.dtype) for z in self.zero_outs
        ]
        out_arrs = self.sharded(*concat_in, *zeros)
        res = []
        for c in range(8):
            res.append(
                {
                    name: np.asarray(out_arrs[i]).reshape(
                        8, *self.out_avals[i].shape
                    )[c]
                    for i, name in enumerate(self.out_names)
                }
            )
        return res


_RUNNER = None


def _get_runner():
    global _RUNNER
    if _RUNNER is None:
        _RUNNER = _Runner()
    return _RUNNER


def make_in_maps(inputs):
    f32 = lambda x: np.ascontiguousarray(np.asarray(x, dtype=np.float32))
    q, k, v = f32(inputs["q"]), f32(inputs["k"]), f32(inputs["v"])
    Wq, Wk, Wv, Wo = (f32(inputs[n]) for n in ("Wq", "Wk", "Wv", "Wo"))
    bq, bk, bv = (f32(inputs[n]) for n in ("bq", "bk", "bv"))
    in_maps = []
    for c in range(8):
        b, mh = c // 2, c % 2
        sl = slice(mh * M, (mh + 1) * M)
        in_maps.append(
            {
                "qb": q[b],
                "kb": np.ascontiguousarray(k[b, sl]),
                "vb": np.ascontiguousarray(v[b, sl]),
                "Wq": Wq, "Wk": Wk, "Wv": Wv, "Wo": Wo,
                "bq": bq, "bk": bk, "bv": bv,
            }
        )
    return in_maps


def assemble_out(results, bo):
    out = np.empty((B, N, HIDDEN), dtype=np.float32)
    for b in range(B):
        acc = results[2 * b]["outT"] + results[2 * b + 1]["outT"]
        out[b] = acc.T + bo[None, :]
    return out


def kernel(**inputs):
    runner = _get_runner()
    res = runner.run(runner.prepare(make_in_maps(inputs)))
    bo = np.asarray(inputs["bo"], dtype=np.float32)
    return assemble_out(res, bo)
